# revision 1
# baseline (speedup 1.0000x reference)
"""Causal self-attention (RoPE, GQA) on 8 Trainium2 NeuronCores.

Sharding: 2-way data-parallel over batch x 4-way tensor-parallel over heads.
Core c handles batch c//4 and head-group c%4 (4 q-heads, 2 kv-heads).
Each core computes its partial output projection (wo row-shard); the host
sums the 4 partials per batch (the "all-reduce" happens in the unshard step).

Layouts (all transposed so no on-device transposes are ever needed):
  - x is fed as xT [D, S]; Q/K are produced as [head_dim, S] via
    lhsT=weight-slice, rhs=xT; V is produced as [S, dv] via lhsT=xT-slice.
  - RoPE: weight rows are pre-permuted (even components -> rows 0:64,
    odd -> rows 64:128) so rotation is elementwise on row halves.
  - scores are computed transposed [k, q] so softmax numerator feeds the
    PV matmul directly; Z (denominator) via a ones-vector matmul; the
    normalization 1/Z via exp(-ln Z) on the scalar engine plus a K=1
    broadcast matmul.

Scheduling constraint honored throughout: a DVE TensorTensor can carry at
most ONE sync-wait, so every TT here has at most one freshly-produced
cross-engine operand (constants are "warmed" with a dummy DVE touch).
"""

import sys
import numpy as np
import ml_dtypes

sys.path.insert(0, "/opt/trn_rl_repo")

import concourse.bass as bass
import concourse.bacc as bacc
import concourse.mybir as mybir
from concourse import tile
from concourse.bass_utils import run_bass_kernel_spmd

F32 = mybir.dt.float32
F32R = mybir.dt.float32r
BF16 = mybir.dt.bfloat16
AF = mybir.ActivationFunctionType
OP = mybir.AluOpType

B, S, D = 2, 2048, 2048
HQ, HKV, HD = 16, 8, 128
ROPE_THETA = 10000.0
NCORES, TP = 8, 4
HQL, HKL = HQ // TP, HKV // TP        # 4 q heads, 2 kv heads per core
NKT = D // 128                        # 16 contraction tiles
QC = 512                              # q-chunk width
NQC = S // QC                         # 4 q chunks
NSB = S // 128                        # 16 s-blocks
SCALE = 1.0 / float(np.sqrt(HD))
BF = ml_dtypes.bfloat16

USE_ACT_SWAP = False  # fallback path for RoPE if cross-base psum TT fails


def _build_nc(debug=False):
    nc = bacc.Bacc("TRN2", target_bir_lowering=False)

    xT_d = nc.dram_tensor("xT", [NKT, 128, S], F32R, kind="ExternalInput")
    wq_d = nc.dram_tensor("wq_t", [NKT, 128, HQL * HD], F32R, kind="ExternalInput")
    wk_d = nc.dram_tensor("wk_t", [NKT, 128, HKL * HD], F32R, kind="ExternalInput")
    wv_d = nc.dram_tensor("wv_t", [NKT, 128, HKL * HD], F32R, kind="ExternalInput")
    wo_d = nc.dram_tensor("wo_t", [HQL, 128, D], BF16, kind="ExternalInput")
    cos_d = nc.dram_tensor("cos128", [128, S], BF16, kind="ExternalInput")
    sin_d = nc.dram_tensor("sinM", [128, S], BF16, kind="ExternalInput")
    mask_d = nc.dram_tensor("dmask", [2, 128, 2 * QC], BF16, kind="ExternalInput")
    out_d = nc.dram_tensor("out", [NSB, 128, D], F32, kind="ExternalOutput")
    if debug:
        dbg_qt = nc.dram_tensor("dbg_qt", [128, S], BF16, kind="ExternalOutput")
        dbg_kt = nc.dram_tensor("dbg_kt", [128, S], BF16, kind="ExternalOutput")
        dbg_v = nc.dram_tensor("dbg_v", [128, NSB, HKL * HD], BF16, kind="ExternalOutput")
        dbg_ot = nc.dram_tensor("dbg_ot", [128, S], BF16, kind="ExternalOutput")
        dbg_e = nc.dram_tensor("dbg_e", [128, 2, QC], BF16, kind="ExternalOutput")
        dbg_z = nc.dram_tensor("dbg_z", [NQC, 1, QC], F32, kind="ExternalOutput")

    with tile.TileContext(nc) as tc:
        with (
            tc.tile_pool(name="resident", bufs=1) as res,
            tc.tile_pool(name="xstream", bufs=3) as xpool,
            tc.tile_pool(name="ropetmp", bufs=2) as rtmp,
            tc.tile_pool(name="epool", bufs=3) as epool,
            tc.tile_pool(name="small", bufs=2) as small,
            tc.tile_pool(name="outp", bufs=2) as outp,
        ):
            # ---------- resident tiles (weights split per k-tile so the
            # first matmuls start after ~1MB of DMA, not ~12MB) ----------
            wq_sb = [res.tile([128, HQL * HD], F32R, tag=f"wq{kt}", name=f"wq{kt}")
                     for kt in range(NKT)]
            wk_sb = [res.tile([128, HKL * HD], F32R, tag=f"wk{kt}", name=f"wk{kt}")
                     for kt in range(NKT)]
            wv_sb = [res.tile([128, HKL * HD], F32R, tag=f"wv{kt}", name=f"wv{kt}")
                     for kt in range(NKT)]
            wo_sb = res.tile([128, HQL, D], BF16)
            cos_sb = res.tile([128, S], BF16)
            sin_sb = res.tile([128, S], BF16)
            mask_sb = res.tile([128, 2, 2 * QC], BF16)

            ones_bf = res.tile([128, 1], BF16)
            nc.vector.memset(ones_bf[:], 1.0)
            ones_rf = res.tile([1, 128], F32)
            nc.vector.memset(ones_rf[:], 1.0)
            ones_r = res.tile([1, 128], F32R)
            nc.vector.tensor_copy(ones_r[:], ones_rf[:])

            # outputs of phase 1 (resident through phase 2/3)
            qt_sb = [res.tile([128, S], BF16, tag=f"qt{h}", name=f"qt{h}") for h in range(HQL)]
            kt_sb = [res.tile([128, S], BF16, tag=f"kt{h}", name=f"kt{h}") for h in range(HKL)]
            v_sb = res.tile([128, NSB, HKL * HD], BF16)
            ot_sb = [res.tile([128, S], BF16, tag=f"ot{h}", name=f"ot{h}") for h in range(HQL)]

            # ---------- phase 1: QKV projection + RoPE ----------
            with tc.tile_pool(name="ps1", bufs=1, space="PSUM") as ps1:
                for qc in range(NQC):
                    qsl = slice(qc * QC, (qc + 1) * QC)
                    qps = [ps1.tile([128, QC], F32, tag=f"qps{h}", name=f"qps{h}_{qc}") for h in range(HQL)]
                    kps = [ps1.tile([128, QC], F32, tag=f"kps{h}", name=f"kps{h}_{qc}") for h in range(HKL)]
                    vps = ps1.tile([128, 4, HKL * HD], F32, tag="vps")
                    for kt in range(NKT):
                        if qc == 0:
                            # stream weights alongside the first x chunk on the
                            # second HWDGE queue (scalar) so they don't delay x
                            nc.scalar.dma_start(wq_sb[kt][:], wq_d[kt, :, :])
                            nc.scalar.dma_start(wk_sb[kt][:], wk_d[kt, :, :])
                            nc.scalar.dma_start(wv_sb[kt][:], wv_d[kt, :, :])
                        xt = xpool.tile([128, QC], F32R)
                        nc.sync.dma_start(xt[:], xT_d[kt, :, qsl])
                        if qc == 0 and kt == 8:
                            # tables needed from the first drain onward
                            nc.scalar.dma_start(cos_sb[:], cos_d[:])
                            nc.scalar.dma_start(sin_sb[:], sin_d[:])
                        if qc == 0 and kt == 12:
                            nc.scalar.dma_start(mask_sb[:],
                                                mask_d.rearrange("g p m -> p g m"))
                            nc.scalar.dma_start(wo_sb[:],
                                                wo_d.rearrange("h p m -> p h m"))
                        st, sp = (kt == 0), (kt == NKT - 1)
                        for h in range(HQL):
                            nc.tensor.matmul(qps[h][:], wq_sb[kt][:, h * HD:(h + 1) * HD],
                                             xt[:], start=st, stop=sp)
                        for h in range(HKL):
                            nc.tensor.matmul(kps[h][:], wk_sb[kt][:, h * HD:(h + 1) * HD],
                                             xt[:], start=st, stop=sp)
                        for sb in range(4):
                            # two 256-col outputs share one PSUM bank: only the
                            # bank's first writer may clear has_written (start)
                            nc.tensor.matmul(vps[:, sb, :], xt[:, sb * 128:(sb + 1) * 128],
                                             wv_sb[kt][:],
                                             start=(st and sb % 2 == 0), stop=sp,
                                             skip_group_check=True)

                    # drain: one fast ACT copy per tile frees the PSUM bank;
                    # RoPE then runs SBUF-side in bf16 on the DVE fast modes
                    evacs = []
                    for i, (ps, dst) in enumerate(
                            [(qps[i], qt_sb[i]) for i in range(HQL)]
                            + [(kps[i], kt_sb[i]) for i in range(HKL)]):
                        qsb = rtmp.tile([128, QC], BF16, tag="evac",
                                        name=f"evac{qc}_{i}")
                        nc.scalar.copy(qsb[:], ps[:])
                        evacs.append((qsb, dst))
                    for sb in range(4):
                        nc.scalar.copy(v_sb[:, qc * 4 + sb, :], vps[:, sb, :])
                    for i, (qsb, dst) in enumerate(evacs):
                        qsw = rtmp.tile([128, QC], BF16, tag="swap",
                                        name=f"swap{qc}_{i}")
                        nc.vector.tensor_copy(qsw[0:64, :], qsb[64:128, :])
                        nc.vector.tensor_copy(qsw[64:128, :], qsb[0:64, :])
                        a_t = rtmp.tile([128, QC], BF16, tag="ropeA")
                        nc.vector.tensor_tensor(a_t[:], qsb[:], cos_sb[:, qsl], OP.mult)
                        b_t = rtmp.tile([128, QC], BF16, tag="ropeB")
                        nc.vector.tensor_tensor(b_t[:], qsw[:], sin_sb[:, qsl], OP.mult)
                        nc.vector.tensor_tensor(dst[:, qsl], a_t[:], b_t[:], OP.add)

            if debug:
                nc.sync.dma_start(dbg_qt[:], qt_sb[0][:])
                nc.sync.dma_start(dbg_kt[:], kt_sb[0][:])
                nc.sync.dma_start(dbg_v[:], v_sb[:])

            # ---------- phase 2: attention (+ interleaved output proj) ----------
            with (
                tc.tile_pool(name="ps2s", bufs=2, space="PSUM") as ps2s,
                tc.tile_pool(name="ps2o", bufs=1, space="PSUM") as ps2o,
            ):
                for qc in range(NQC):
                    for h in range(HQL):
                        kv = h // 2
                        qsl = slice(qc * QC, (qc + 1) * QC)
                        ops_t = ps2o.tile([128, QC], F32, tag="opv")
                        zps_t = ps2o.tile([1, QC], F32, tag="zps")
                        ngrp = 2 * qc + 2          # groups of 2 k-blocks
                        for g in range(ngrp):
                            diag = (g >= 2 * qc)   # last two groups touch diagonal
                            sps = ps2s.tile([128, 2, QC], F32, tag="sps")
                            e_t = epool.tile([128, 2, QC], BF16, tag="etile")
                            for j in range(2):
                                kb = 2 * g + j
                                off = (kb - 4 * qc) * 128 if kb >= 4 * qc else 0
                                nc.tensor.matmul(
                                    sps[:, j, off:], kt_sb[kv][:, kb * 128:(kb + 1) * 128],
                                    qt_sb[h][:, qc * QC + off:(qc + 1) * QC],
                                    start=True, stop=True)
                            nc.scalar.activation(
                                e_t[:].rearrange("p a b -> p (a b)"),
                                sps[:].rearrange("p a b -> p (a b)"),
                                AF.Exp, scale=SCALE)
                            if debug and h == 0 and qc == 3 and g == 0:
                                nc.sync.dma_start(dbg_e[:], e_t[:])
                            if diag:
                                gi = g - 2 * qc
                                nc.vector.tensor_tensor(
                                    e_t[:].rearrange("p a b -> p (a b)"),
                                    e_t[:].rearrange("p a b -> p (a b)"),
                                    mask_sb[:, gi, :], OP.mult)
                            for j in range(2):
                                kb = 2 * g + j
                                off = (kb - 4 * qc) * 128 if kb >= 4 * qc else 0
                                st = (kb == 0)
                                sp = (kb == 4 * qc + 3)
                                nc.tensor.matmul(
                                    ops_t[:, off:], v_sb[:, kb, kv * HD:(kv + 1) * HD],
                                    e_t[:, j, off:], start=st, stop=sp,
                                    skip_group_check=True)
                                nc.tensor.matmul(
                                    zps_t[:, off:], ones_bf[:], e_t[:, j, off:],
                                    start=st, stop=sp, skip_group_check=True)

                        # 1/Z then broadcast, then normalize O^T
                        z_sb = small.tile([1, QC], F32, tag="zsb")
                        nc.vector.tensor_copy(z_sb[:], zps_t[:])
                        if debug and h == 0:
                            nc.sync.dma_start(dbg_z[qc, :, :], z_sb[:])
                        rz = small.tile([1, QC], F32, tag="rz")
                        nc.vector.reciprocal_approx_fast(rz[:], z_sb[:])
                        rz_r = small.tile([1, QC], F32R, tag="rzr")
                        nc.vector.tensor_copy(rz_r[:], rz[:])
                        rb_ps = ps2s.tile([128, QC], F32, tag="rbps")
                        nc.tensor.matmul(rb_ps[:], ones_r[:], rz_r[:],
                                         start=True, stop=True)
                        rb_sb = small.tile([128, QC], F32, tag="rbsb")
                        nc.vector.tensor_copy(rb_sb[:], rb_ps[:])
                        nc.vector.tensor_tensor(ot_sb[h][:, qsl], ops_t[:], rb_sb[:],
                                                OP.mult)

                    # output projection for this qc's 4 s-blocks (all heads'
                    # O^T columns for them were just produced)
                    for sb in range(4 * qc, 4 * qc + 4):
                        for dcp in range(2):
                            fps = ps2s.tile([128, 2, QC], F32, tag="sps",
                                            name=f"fps{sb}_{dcp}")
                            for j in range(2):
                                dc = 2 * dcp + j
                                for h in range(HQL):
                                    nc.tensor.matmul(
                                        fps[:, j, :],
                                        ot_sb[h][:, sb * 128:(sb + 1) * 128],
                                        wo_sb[:, h, dc * QC:(dc + 1) * QC],
                                        start=(h == 0), stop=(h == HQL - 1))
                            o_sb = outp.tile([128, 2, QC], F32, tag="osb")
                            nc.scalar.copy(o_sb[:].rearrange("p a b -> p (a b)"),
                                           fps[:].rearrange("p a b -> p (a b)"))
                            nc.sync.dma_start(
                                out_d[sb, :, dcp * 2 * QC:(dcp + 1) * 2 * QC],
                                o_sb[:].rearrange("p a b -> p (a b)"))

            if debug:
                nc.sync.dma_start(dbg_ot[:], ot_sb[0][:])

    nc.compile()
    return nc


_NC_CACHE = None


def _get_nc():
    global _NC_CACHE
    if _NC_CACHE is None:
        _NC_CACHE = _build_nc()
    return _NC_CACHE


def _rope_tables():
    inv = 1.0 / (ROPE_THETA ** (np.arange(0, HD, 2, dtype=np.float64) / HD))  # [64]
    t = np.arange(S, dtype=np.float64)
    ang = np.outer(inv, t)                      # [64, S]
    cos = np.cos(ang).astype(np.float32)
    sin = np.sin(ang).astype(np.float32)
    cos128 = np.concatenate([cos, cos], axis=0).astype(BF)  # [128, S]
    sinM = np.concatenate([-sin, sin], axis=0).astype(BF)
    return cos128, sinM


def _masks():
    # dmask[g] covers a group of 2 k-blocks at diagonal offsets (2g*128,(2g+1)*128)
    q = np.arange(QC)
    m = np.zeros((2, 128, 2 * QC), np.float32)
    for g in range(2):
        for j in range(2):
            off = (2 * g + j) * 128
            k = np.arange(128) + off
            m[g, :, j * QC:(j + 1) * QC] = (k[:, None] <= q[None, :])
    return m.astype(BF)


def prepare_inputs(x, wq, wk, wv, wo):
    """Build the 8 per-core input dicts from full inputs."""
    perm = np.concatenate([np.arange(0, HD, 2), np.arange(1, HD, 2)])
    cos128, sinM = _rope_tables()
    dmask = _masks()

    x = np.asarray(x, np.float32)
    wq = np.asarray(wq, np.float32).reshape(HQ, HD, D)[:, perm, :]
    wk = np.asarray(wk, np.float32).reshape(HKV, HD, D)[:, perm, :]
    wv = np.asarray(wv, np.float32).reshape(HKV, HD, D)
    wo = np.asarray(wo, np.float32)              # [D, HQ*HD]

    in_maps = []
    for c in range(NCORES):
        b, hg = divmod(c, TP)
        qh = slice(hg * HQL, (hg + 1) * HQL)
        kh = slice(hg * HKL, (hg + 1) * HKL)
        xT = np.ascontiguousarray(x[b].T).reshape(NKT, 128, S)
        wq_t = np.ascontiguousarray(
            wq[qh].reshape(HQL * HD, D).T).reshape(NKT, 128, HQL * HD)
        wk_t = np.ascontiguousarray(
            wk[kh].reshape(HKL * HD, D).T).reshape(NKT, 128, HKL * HD)
        wv_t = np.ascontiguousarray(
            wv[kh].reshape(HKL * HD, D).T).reshape(NKT, 128, HKL * HD)
        wo_t = np.ascontiguousarray(
            wo[:, hg * HQL * HD:(hg + 1) * HQL * HD].T.reshape(HQL, HD, D)
        ).astype(BF)
        in_maps.append({
            "xT": xT, "wq_t": wq_t, "wk_t": wk_t, "wv_t": wv_t, "wo_t": wo_t,
            "cos128": cos128, "sinM": sinM, "dmask": dmask,
        })
    return in_maps


def _install_ntff_hook():
    """The agent image's antenv lacks axon_hooks; synthesize it so
    run_bass_kernel_spmd(trace=True) can capture NTFF profiles."""
    import sys as _sys
    import types, contextlib, ctypes

    if "antenv.axon_hooks" in _sys.modules:
        return
    so_path = "/opt/axon/libaxon_pjrt.so"
    lib = ctypes.CDLL(so_path)
    if not hasattr(lib, "axon_start_nrt_profile"):
        return
    lib.axon_start_nrt_profile.argtypes = [ctypes.POINTER(ctypes.c_int64),
                                           ctypes.c_size_t]
    lib.axon_start_nrt_profile.restype = ctypes.c_int64
    lib.axon_stop_nrt_profile.argtypes = [ctypes.c_char_p]
    lib.axon_stop_nrt_profile.restype = ctypes.c_int64

    @contextlib.contextmanager
    def _hook(output_dir, device_ids):
        import jax
        jax.devices()
        if device_ids:
            ids = (ctypes.c_int64 * len(device_ids))(*device_ids)
            rc = lib.axon_start_nrt_profile(ids, len(device_ids))
        else:
            rc = lib.axon_start_nrt_profile(None, 0)
        if rc != 0:
            raise RuntimeError(f"axon_start_nrt_profile rc={rc}")
        try:
            yield
        finally:
            n = lib.axon_stop_nrt_profile(str(output_dir).encode())
            print(f"ntff profile: {n} file(s) written to {output_dir}",
                  file=_sys.stderr)

    mod = types.ModuleType("antenv.axon_hooks")
    mod.get_axon_ntff_profile_hook = lambda: _hook
    mod.set_axon_ntff_profile_hook = lambda h: None
    _sys.modules["antenv.axon_hooks"] = mod
    try:
        import antenv
        antenv.axon_hooks = mod
    except ImportError:
        pass


def kernel(x, wq, wk, wv, wo, _trace=False, _trace_cores=None):
    in_maps = prepare_inputs(x, wq, wk, wv, wo)
    if _trace:
        _install_ntff_hook()
    nc = _get_nc()
    res = run_bass_kernel_spmd(
        nc, in_maps, core_ids=list(range(NCORES)),
        trace=_trace, trace_cores=_trace_cores)
    out = np.zeros((B, S, D), np.float32)
    for c in range(NCORES):
        b = c // TP
        out[b] += res.results[c]["out"].reshape(S, D)
    kernel.last_results = res
    return out


if __name__ == "__main__":
    rng = np.random.default_rng(0)
    x = rng.standard_normal((B, S, D), dtype=np.float32)
    sc = 1.0 / np.sqrt(D)
    wq = (rng.standard_normal((HQ * HD, D), dtype=np.float32) * sc)
    wk = (rng.standard_normal((HKV * HD, D), dtype=np.float32) * sc)
    wv = (rng.standard_normal((HKV * HD, D), dtype=np.float32) * sc)
    wo = (rng.standard_normal((D, HQ * HD), dtype=np.float32) * sc)
    out = kernel(x, wq, wk, wv, wo)
    print("ran", out.shape, out.dtype, float(np.abs(out).mean()))



# revision 6
# speedup vs baseline: 1.5469x; 1.5469x over previous
"""Causal self-attention (RoPE, GQA) on 8 Trainium2 NeuronCores — v2.

Sharding: 2-way data-parallel over batch x 4-way tensor-parallel over heads.
Core c handles batch c//4 and head-group c%4 (4 q-heads, 2 kv-heads).
Each core computes its partial output projection (wo row-shard); the host
sums the 4 partials per batch (the "all-reduce" happens in the unshard step).

v2 changes vs v1 (518us):
  - everything bf16 (x, weights): 1 cyc/row matmuls + FWL weight loads,
    half the DMA bytes.
  - moving dim 1024 everywhere possible (bf16 max), halving MM count.
  - GQA head pairing: the 2 q-heads sharing a kv head are batched in one
    matmul (rhs [128, 2, n]) for scores / PV / Z.
  - attention is software-pipelined: scores(i+1) issued before PV/Z(i) so
    the PE never waits for the exp (ACT) of the current block.
  - Z (softmax denominator) via ones-MATRIX matmul -> [128, 2, 512] psum:
    the denominator is broadcast to all partitions by the PE itself, so
    normalization is one DVE reciprocal + one TT (no K=1 broadcast matmul).
  - V computed transposed like K (good LDWEIGHTS amortization) then
    PE-transposed in 128x128 blocks (cheap) into PV layout.
  - output projection as a dense phase-3 with full PSUM double-buffering.

Layouts:
  - x fed as xT [D, S] bf16; Q/K produced as [head_dim, S] via lhsT=weight
    slice; V^T the same way, then transposed.
  - RoPE: weight rows pre-permuted (even components -> rows 0:64, odd ->
    64:128) so rotation is elementwise on row halves.
  - scores computed transposed [k, q-pair] so the softmax numerator feeds
    the PV matmul directly.
"""

import sys
import numpy as np
import ml_dtypes

sys.path.insert(0, "/opt/trn_rl_repo")

import concourse.bass as bass
import concourse.bacc as bacc
import concourse.mybir as mybir
from concourse import tile
from concourse.bass_utils import run_bass_kernel_spmd

F32 = mybir.dt.float32
BF16 = mybir.dt.bfloat16
AF = mybir.ActivationFunctionType
OP = mybir.AluOpType

B, S, D = 2, 2048, 2048
HQ, HKV, HD = 16, 8, 128
ROPE_THETA = 10000.0
NCORES, TP = 8, 4
HQL, HKL = HQ // TP, HKV // TP        # 4 q heads, 2 kv heads per core
NKT = D // 128                        # 16 contraction tiles
NSB = S // 128                        # 16 s-blocks
QC = 512                              # q-chunk width in attention
NQC = S // QC                         # 4 q chunks
SH = S // 2                           # phase-1 S-half (moving dim 1024)
SCALE = 1.0 / float(np.sqrt(HD))
BF = ml_dtypes.bfloat16


def _build_nc():
    nc = bacc.Bacc("TRN2", target_bir_lowering=False)

    xT_d = nc.dram_tensor("xT", [NKT, 128, S], BF16, kind="ExternalInput")
    wq_d = nc.dram_tensor("wq_t", [NKT, 128, HQL * HD], BF16, kind="ExternalInput")
    wk_d = nc.dram_tensor("wk_t", [NKT, 128, HKL * HD], BF16, kind="ExternalInput")
    wv_d = nc.dram_tensor("wv_t", [NKT, 128, HKL * HD], BF16, kind="ExternalInput")
    wo_d = nc.dram_tensor("wo_t", [128, HQL, D], BF16, kind="ExternalInput")
    cos_d = nc.dram_tensor("cos128", [128, S], BF16, kind="ExternalInput")
    sin_d = nc.dram_tensor("sinM", [128, S], BF16, kind="ExternalInput")
    mask_d = nc.dram_tensor("mask128", [128, 128], BF16, kind="ExternalInput")
    id_d = nc.dram_tensor("ident128", [128, 128], BF16, kind="ExternalInput")
    out_d = nc.dram_tensor("out", [NSB, 128, D], F32, kind="ExternalOutput")

    with tile.TileContext(nc) as tc:
        with tc.tile_pool(name="res", bufs=1) as res:
            # residents that live across phases
            qt2 = [res.tile([128, 2, S], BF16, name=f"qt2_{p}") for p in range(2)]
            kt2 = res.tile([128, 2, S], BF16, name="kt2")
            v_sb = res.tile([128, NSB, HKL * HD], BF16, name="v_sb")
            ot2 = [res.tile([128, 2, S], BF16, name=f"ot2_{p}") for p in range(2)]
            wo_sb = res.tile([128, HQL, D], BF16, name="wo_sb")
            cos_sb = res.tile([128, S], BF16, name="cos_sb")
            sin_sb = res.tile([128, S], BF16, name="sin_sb")
            mask_sb = res.tile([128, 128], BF16, name="mask_sb")
            id_sb = res.tile([128, 128], BF16, name="id_sb")
            ones_sb = res.tile([128, 128], BF16, name="ones_sb")
            warm_sb = res.tile([1, 128], BF16, name="warm_sb")

            nc.vector.memset(ones_sb[:], 1.0)

            # ---------- phase 1: QKV^T projection + RoPE + V transpose ----
            with (
                tc.tile_pool(name="xw", bufs=1) as xw,
                tc.tile_pool(name="rtmp", bufs=2) as rtmp,
                tc.tile_pool(name="ps1", bufs=2, space="PSUM") as ps1,
                tc.tile_pool(name="tps", bufs=2, space="PSUM") as tps,
            ):
                xT_sb = [xw.tile([128, S], BF16, tag=f"x{kt}", name=f"x{kt}")
                         for kt in range(NKT)]
                wq_sb = [xw.tile([128, HQL * HD], BF16, tag=f"wq{kt}", name=f"wq{kt}")
                         for kt in range(NKT)]
                wk_sb = [xw.tile([128, HKL * HD], BF16, tag=f"wk{kt}", name=f"wk{kt}")
                         for kt in range(NKT)]
                wv_sb = [xw.tile([128, HKL * HD], BF16, tag=f"wv{kt}", name=f"wv{kt}")
                         for kt in range(NKT)]
                vt_sb = xw.tile([128, 2, S], BF16, name="vt_sb")

                # tables first (small) on the scalar queue, then weights;
                # x tiles stream on the sync queue.
                nc.scalar.dma_start(cos_sb[:], cos_d[:])
                nc.scalar.dma_start(sin_sb[:], sin_d[:])
                nc.scalar.dma_start(mask_sb[:], mask_d[:])
                nc.scalar.dma_start(id_sb[:], id_d[:])
                for kt in range(NKT):
                    nc.scalar.dma_start(wk_sb[kt][:], wk_d[kt, :, :])
                for kt in range(NKT):
                    nc.scalar.dma_start(wq_sb[kt][:], wq_d[kt, :, :])
                for kt in range(NKT):
                    nc.scalar.dma_start(wv_sb[kt][:], wv_d[kt, :, :])
                nc.scalar.dma_start(wo_sb[:], wo_d[:])
                for kt in range(NKT):
                    nc.sync.dma_start(xT_sb[kt][:], xT_d[kt, :, :])

                # warm the DVE against the table DMAs so later TTs carry a
                # single fresh cross-engine wait only.
                nc.vector.tensor_copy(warm_sb[:], cos_sb[0:1, 0:128])
                nc.vector.tensor_copy(warm_sb[:], sin_sb[0:1, 0:128])
                nc.vector.tensor_copy(warm_sb[:], mask_sb[0:1, 0:128])

                def rope_drain(ps, dst, hsl, tag):
                    qsb = rtmp.tile([128, SH], BF16, tag="evac", name=f"ev{tag}")
                    nc.scalar.copy(qsb[:], ps[:])
                    qsw = rtmp.tile([128, SH], BF16, tag="swap", name=f"sw{tag}")
                    nc.vector.tensor_copy(qsw[0:64, :], qsb[64:128, :])
                    nc.vector.tensor_copy(qsw[64:128, :], qsb[0:64, :])
                    a_t = rtmp.tile([128, SH], BF16, tag="ropeA", name=f"ra{tag}")
                    nc.vector.tensor_tensor(a_t[:], qsb[:], cos_sb[:, hsl], OP.mult)
                    b_t = rtmp.tile([128, SH], BF16, tag="ropeB", name=f"rb{tag}")
                    nc.vector.tensor_tensor(b_t[:], qsw[:], sin_sb[:, hsl], OP.mult)
                    nc.vector.tensor_tensor(dst, a_t[:], b_t[:], OP.add)

                # runs: (kind, weight-col j, S-half h)
                runs = []
                for h in range(2):
                    runs += [("k", j, h) for j in range(HKL)]
                runs += [("q", j, h) for j in range(HQL) for h in range(2)]
                runs += [("v", j, h) for h in range(2) for j in range(HKL)]

                for kind, j, h in runs:
                    hsl = slice(h * SH, (h + 1) * SH)
                    w_list = {"q": wq_sb, "k": wk_sb, "v": wv_sb}[kind]
                    ps = ps1.tile([128, SH], F32, tag="ps", name=f"ps_{kind}{j}{h}")
                    for kt in range(NKT):
                        # two 512-wide chunks = two PSUM banks, alternated so
                        # each MM's drain overlaps the next MM's fill
                        for c in range(2):
                            nc.tensor.matmul(
                                ps[:, c * 512:(c + 1) * 512],
                                w_list[kt][:, j * HD:(j + 1) * HD],
                                xT_sb[kt][:, h * SH + c * 512:
                                          h * SH + (c + 1) * 512],
                                start=(kt == 0), stop=(kt == NKT - 1),
                                skip_group_check=True)
                    if kind == "q":
                        rope_drain(ps, qt2[j // 2][:, j % 2, hsl], hsl, f"q{j}{h}")
                    elif kind == "k":
                        rope_drain(ps, kt2[:, j, hsl], hsl, f"k{j}{h}")
                    else:
                        nc.scalar.copy(vt_sb[:, j, hsl], ps[:])
                        tp = tps.tile([128, 8, 128], BF16, tag="tp",
                                      name=f"tp{j}{h}")
                        for blk in range(8):
                            nc.tensor.transpose(
                                tp[:, blk, :],
                                vt_sb[:, j, h * SH + blk * 128:
                                      h * SH + (blk + 1) * 128],
                                id_sb[:])
                        nc.scalar.copy(
                            v_sb[:, h * 8:h * 8 + 8, j * HD:(j + 1) * HD],
                            tp[:])

            # ---------- phase 2: attention, softmax-pipelined ----------
            tasks = []
            for qc in range(NQC):
                for p in range(2):
                    nblk = 4 * qc + 4
                    for kb in range(nblk):
                        diag = kb >= 4 * qc
                        doff = (kb - 4 * qc) * 128 if diag else 0
                        tasks.append(dict(
                            p=p, qc=qc, kb=kb, diag=diag, doff=doff,
                            first=(kb == 0), last=(kb == nblk - 1)))

            with (
                tc.tile_pool(name="psS", bufs=2, space="PSUM") as psS,
                tc.tile_pool(name="psO", bufs=1, space="PSUM") as psO,
                tc.tile_pool(name="psZ", bufs=1, space="PSUM") as psZ,
                tc.tile_pool(name="epool", bufs=3) as epool,
                tc.tile_pool(name="opool", bufs=2) as opool,
                tc.tile_pool(name="rzpool", bufs=2) as rzpool,
            ):
                live = {}   # per-(p,qc) psum accumulators
                deferred = []

                def emit_scores(i, t):
                    p, qc, kb, doff = t["p"], t["qc"], t["kb"], t["doff"]
                    sps = psS.tile([128, 2, QC], F32, tag="sps", name=f"sps{i}")
                    t["sps"] = sps
                    for jj in range(2):
                        nc.tensor.matmul(
                            sps[:, jj, doff:],
                            kt2[:, p, kb * 128:(kb + 1) * 128],
                            qt2[p][:, jj, qc * QC + doff:(qc + 1) * QC],
                            start=True, stop=True, skip_group_check=True)

                def emit_expmask(i, t):
                    doff = t["doff"]
                    e2 = epool.tile([128, 2, QC], BF16, tag="e", name=f"e{i}")
                    t["e2"] = e2
                    sps = t["sps"]
                    if t["diag"]:
                        nc.scalar.activation(
                            e2[:, :, doff:doff + 128], sps[:, :, doff:doff + 128],
                            AF.Exp, scale=SCALE)
                        for jj in range(2):
                            nc.vector.tensor_tensor(
                                e2[:, jj, doff:doff + 128],
                                e2[:, jj, doff:doff + 128],
                                mask_sb[:], OP.mult)
                        if doff + 128 < QC:
                            nc.scalar.activation(
                                e2[:, :, doff + 128:], sps[:, :, doff + 128:],
                                AF.Exp, scale=SCALE)
                    else:
                        nc.scalar.activation(e2[:], sps[:], AF.Exp, scale=SCALE)

                def emit_pvz(i, t):
                    p, qc, kb, doff = t["p"], t["qc"], t["kb"], t["doff"]
                    if t["first"]:
                        live[(p, qc)] = (
                            psO.tile([128, 2, QC], F32, tag="ops", name=f"ops{i}"),
                            psZ.tile([128, 2, QC], F32, tag="zps", name=f"zps{i}"),
                        )
                    ops, zps = live[(p, qc)]
                    e2 = t["e2"]
                    for jj in range(2):
                        nc.tensor.matmul(
                            ops[:, jj, doff:], v_sb[:, kb, p * HD:(p + 1) * HD],
                            e2[:, jj, doff:], start=t["first"], stop=t["last"],
                            skip_group_check=True)
                    for jj in range(2):
                        nc.tensor.matmul(
                            zps[:, jj, doff:], ones_sb[:],
                            e2[:, jj, doff:], start=t["first"], stop=t["last"],
                            skip_group_check=True)

                def emit_tail(i, t):
                    p, qc = t["p"], t["qc"]
                    ops, zps = live.pop((p, qc))
                    o_un = opool.tile([128, 2, QC], BF16, tag="oun",
                                      name=f"oun{i}")
                    nc.scalar.copy(o_un[:], ops[:])
                    rz = rzpool.tile([128, 2, QC], F32, tag="rz", name=f"rz{i}")
                    nc.vector.reciprocal_approx_fast(rz[:], zps[:])

                    def norm(p=p, qc=qc, o_un=o_un, rz=rz):
                        nc.vector.tensor_tensor(
                            ot2[p][:, :, qc * QC:(qc + 1) * QC],
                            o_un[:], rz[:], OP.mult)
                    deferred.append((i + 2, norm))

                emit_scores(0, tasks[0])
                for i, t in enumerate(tasks):
                    if i + 1 < len(tasks):
                        emit_scores(i + 1, tasks[i + 1])
                    emit_expmask(i, t)
                    emit_pvz(i, t)
                    while deferred and deferred[0][0] <= i:
                        deferred.pop(0)[1]()
                    if t["last"]:
                        emit_tail(i, t)
                for _, fn in deferred:
                    fn()

            # ---------- phase 3: output projection ----------
            with (
                tc.tile_pool(name="ps3", bufs=2, space="PSUM") as ps3,
                tc.tile_pool(name="osb", bufs=2) as osb,
            ):
                for sb in range(NSB):
                    fps = ps3.tile([128, D], F32, tag="fps", name=f"fps{sb}")
                    for p in range(2):
                        for ii in range(2):
                            h = 2 * p + ii
                            lhs = ot2[p][:, ii, sb * 128:(sb + 1) * 128]
                            for dc in range(4):
                                nc.tensor.matmul(
                                    fps[:, dc * 512:(dc + 1) * 512], lhs,
                                    wo_sb[:, h, dc * 512:(dc + 1) * 512],
                                    start=(h == 0), stop=(h == HQL - 1),
                                    skip_group_check=True)
                    o_sb = osb.tile([128, D], F32, tag="osb", name=f"osb{sb}")
                    nc.scalar.copy(o_sb[:], fps[:])
                    nc.sync.dma_start(out_d[sb, :, :], o_sb[:])

    nc.compile()
    return nc


_NC_CACHE = None


def _get_nc():
    global _NC_CACHE
    if _NC_CACHE is None:
        _NC_CACHE = _build_nc()
    return _NC_CACHE


def _rope_tables():
    inv = 1.0 / (ROPE_THETA ** (np.arange(0, HD, 2, dtype=np.float64) / HD))  # [64]
    t = np.arange(S, dtype=np.float64)
    ang = np.outer(inv, t)                      # [64, S]
    cos = np.cos(ang).astype(np.float32)
    sin = np.sin(ang).astype(np.float32)
    cos128 = np.concatenate([cos, cos], axis=0).astype(BF)  # [128, S]
    sinM = np.concatenate([-sin, sin], axis=0).astype(BF)
    return cos128, sinM


def prepare_inputs(x, wq, wk, wv, wo):
    """Build the 8 per-core input dicts from full inputs."""
    perm = np.concatenate([np.arange(0, HD, 2), np.arange(1, HD, 2)])
    cos128, sinM = _rope_tables()
    k_idx = np.arange(128)
    mask128 = (k_idx[:, None] <= k_idx[None, :]).astype(BF)
    ident = np.eye(128, dtype=np.float32).astype(BF)

    x = np.asarray(x, np.float32)
    wq = np.asarray(wq, np.float32).reshape(HQ, HD, D)[:, perm, :]
    wk = np.asarray(wk, np.float32).reshape(HKV, HD, D)[:, perm, :]
    wv = np.asarray(wv, np.float32)              # [HKV*HD, D]
    wo = np.asarray(wo, np.float32)              # [D, HQ*HD]

    in_maps = []
    for c in range(NCORES):
        b, hg = divmod(c, TP)
        qh = slice(hg * HQL, (hg + 1) * HQL)
        kh = slice(hg * HKL, (hg + 1) * HKL)
        xT = np.ascontiguousarray(x[b].T).reshape(NKT, 128, S).astype(BF)
        wq_t = np.ascontiguousarray(
            wq[qh].reshape(HQL * HD, D).T).reshape(NKT, 128, HQL * HD).astype(BF)
        wk_t = np.ascontiguousarray(
            wk[kh].reshape(HKL * HD, D).T).reshape(NKT, 128, HKL * HD).astype(BF)
        wv_t = np.ascontiguousarray(
            wv[hg * HKL * HD:(hg + 1) * HKL * HD].T
        ).reshape(NKT, 128, HKL * HD).astype(BF)
        wo_t = np.ascontiguousarray(
            wo[:, hg * HQL * HD:(hg + 1) * HQL * HD].T.reshape(HQL, 128, D)
            .transpose(1, 0, 2)).astype(BF)
        in_maps.append({
            "xT": xT, "wq_t": wq_t, "wk_t": wk_t, "wv_t": wv_t, "wo_t": wo_t,
            "cos128": cos128, "sinM": sinM, "mask128": mask128,
            "ident128": ident,
        })
    return in_maps


def _install_ntff_hook():
    """The agent image's antenv lacks axon_hooks; synthesize it so
    run_bass_kernel_spmd(trace=True) can capture NTFF profiles."""
    import sys as _sys
    import types, contextlib, ctypes

    if "antenv.axon_hooks" in _sys.modules:
        return
    so_path = "/opt/axon/libaxon_pjrt.so"
    lib = ctypes.CDLL(so_path)
    if not hasattr(lib, "axon_start_nrt_profile"):
        return
    lib.axon_start_nrt_profile.argtypes = [ctypes.POINTER(ctypes.c_int64),
                                           ctypes.c_size_t]
    lib.axon_start_nrt_profile.restype = ctypes.c_int64
    lib.axon_stop_nrt_profile.argtypes = [ctypes.c_char_p]
    lib.axon_stop_nrt_profile.restype = ctypes.c_int64

    @contextlib.contextmanager
    def _hook(output_dir, device_ids):
        import jax
        jax.devices()
        if device_ids:
            ids = (ctypes.c_int64 * len(device_ids))(*device_ids)
            rc = lib.axon_start_nrt_profile(ids, len(device_ids))
        else:
            rc = lib.axon_start_nrt_profile(None, 0)
        if rc != 0:
            raise RuntimeError(f"axon_start_nrt_profile rc={rc}")
        try:
            yield
        finally:
            n = lib.axon_stop_nrt_profile(str(output_dir).encode())
            print(f"ntff profile: {n} file(s) written to {output_dir}",
                  file=_sys.stderr)

    mod = types.ModuleType("antenv.axon_hooks")
    mod.get_axon_ntff_profile_hook = lambda: _hook
    mod.set_axon_ntff_profile_hook = lambda h: None
    _sys.modules["antenv.axon_hooks"] = mod
    try:
        import antenv
        antenv.axon_hooks = mod
    except ImportError:
        pass


def kernel(x, wq, wk, wv, wo, _trace=False, _trace_cores=None):
    in_maps = prepare_inputs(x, wq, wk, wv, wo)
    if _trace:
        _install_ntff_hook()
    nc = _get_nc()
    res = run_bass_kernel_spmd(
        nc, in_maps, core_ids=list(range(NCORES)),
        trace=_trace, trace_cores=_trace_cores)
    out = np.zeros((B, S, D), np.float32)
    for c in range(NCORES):
        b = c // TP
        out[b] += res.results[c]["out"].reshape(S, D)
    kernel.last_results = res
    return out


if __name__ == "__main__":
    rng = np.random.default_rng(0)
    x = rng.standard_normal((B, S, D), dtype=np.float32)
    sc = 1.0 / np.sqrt(D)
    wq = (rng.standard_normal((HQ * HD, D), dtype=np.float32) * sc)
    wk = (rng.standard_normal((HKV * HD, D), dtype=np.float32) * sc)
    wv = (rng.standard_normal((HKV * HD, D), dtype=np.float32) * sc)
    wo = (rng.standard_normal((D, HQ * HD), dtype=np.float32) * sc)
    out = kernel(x, wq, wk, wv, wo)
    print("ran", out.shape, out.dtype, float(np.abs(out).mean()))


# revision 12
# speedup vs baseline: 1.5778x; 1.0200x over previous
"""Causal self-attention (RoPE, GQA) on 8 Trainium2 NeuronCores — v2.

Sharding: 2-way data-parallel over batch x 4-way tensor-parallel over heads.
Core c handles batch c//4 and head-group c%4 (4 q-heads, 2 kv-heads).
Each core computes its partial output projection (wo row-shard); the host
sums the 4 partials per batch (the "all-reduce" happens in the unshard step).

v2 changes vs v1 (518us):
  - everything bf16 (x, weights): 1 cyc/row matmuls + FWL weight loads,
    half the DMA bytes.
  - moving dim 1024 everywhere possible (bf16 max), halving MM count.
  - GQA head pairing: the 2 q-heads sharing a kv head are batched in one
    matmul (rhs [128, 2, n]) for scores / PV / Z.
  - attention is software-pipelined: scores(i+1) issued before PV/Z(i) so
    the PE never waits for the exp (ACT) of the current block.
  - Z (softmax denominator) via ones-MATRIX matmul -> [128, 2, 512] psum:
    the denominator is broadcast to all partitions by the PE itself, so
    normalization is one DVE reciprocal + one TT (no K=1 broadcast matmul).
  - V computed transposed like K (good LDWEIGHTS amortization) then
    PE-transposed in 128x128 blocks (cheap) into PV layout.
  - output projection as a dense phase-3 with full PSUM double-buffering.

Layouts:
  - x fed as xT [D, S] bf16; Q/K produced as [head_dim, S] via lhsT=weight
    slice; V^T the same way, then transposed.
  - RoPE: weight rows pre-permuted (even components -> rows 0:64, odd ->
    64:128) so rotation is elementwise on row halves.
  - scores computed transposed [k, q-pair] so the softmax numerator feeds
    the PV matmul directly.
"""

import sys
import numpy as np
import ml_dtypes

sys.path.insert(0, "/opt/trn_rl_repo")

import concourse.bass as bass
import concourse.bacc as bacc
import concourse.mybir as mybir
from concourse import tile
from concourse.bass_utils import run_bass_kernel_spmd

F32 = mybir.dt.float32
BF16 = mybir.dt.bfloat16
AF = mybir.ActivationFunctionType
OP = mybir.AluOpType

B, S, D = 2, 2048, 2048
HQ, HKV, HD = 16, 8, 128
ROPE_THETA = 10000.0
NCORES, TP = 8, 4
HQL, HKL = HQ // TP, HKV // TP        # 4 q heads, 2 kv heads per core
NKT = D // 128                        # 16 contraction tiles
NSB = S // 128                        # 16 s-blocks
QC = 512                              # q-chunk width in attention
NQC = S // QC                         # 4 q chunks
SH = S // 2                           # phase-1 S-half (moving dim 1024)
SCALE = 1.0 / float(np.sqrt(HD))
BF = ml_dtypes.bfloat16


def _build_nc():
    nc = bacc.Bacc("TRN2", target_bir_lowering=False)

    xT_d = nc.dram_tensor("xT", [NKT, 128, S], BF16, kind="ExternalInput")
    wq_d = nc.dram_tensor("wq_t", [NKT, 128, HQL * HD], BF16, kind="ExternalInput")
    wk_d = nc.dram_tensor("wk_t", [NKT, 128, HKL * HD], BF16, kind="ExternalInput")
    wv_d = nc.dram_tensor("wv_t", [NKT, 128, HKL * HD], BF16, kind="ExternalInput")
    wo_d = nc.dram_tensor("wo_t", [128, HQL, D], BF16, kind="ExternalInput")
    cos_d = nc.dram_tensor("cos128", [128, S], BF16, kind="ExternalInput")
    sin_d = nc.dram_tensor("sinM", [128, S], BF16, kind="ExternalInput")
    mask_d = nc.dram_tensor("mask128", [128, 128], BF16, kind="ExternalInput")
    id_d = nc.dram_tensor("ident128", [128, 128], BF16, kind="ExternalInput")
    out_d = nc.dram_tensor("out", [NSB, 128, D], BF16, kind="ExternalOutput")

    with tile.TileContext(nc) as tc:
        with tc.tile_pool(name="res", bufs=1) as res:
            # residents that live across phases
            qt2 = [res.tile([128, 2, S], BF16, name=f"qt2_{p}") for p in range(2)]
            kt2 = res.tile([128, 2, S], BF16, name="kt2")
            v_sb = res.tile([128, NSB, HKL * HD], BF16, name="v_sb")
            ot2 = [res.tile([128, 2, S], BF16, name=f"ot2_{p}") for p in range(2)]
            wo_sb = res.tile([128, HQL, D], BF16, name="wo_sb")
            cos_sb = res.tile([128, S], BF16, name="cos_sb")
            sin_sb = res.tile([128, S], BF16, name="sin_sb")
            mask_sb = res.tile([128, 128], BF16, name="mask_sb")
            id_sb = res.tile([128, 128], BF16, name="id_sb")
            ones_sb = res.tile([128, 128], BF16, name="ones_sb")
            warm_sb = res.tile([1, 128], BF16, name="warm_sb")

            nc.vector.memset(ones_sb[:], 1.0)

            # ---------- phase 1: QKV^T projection + RoPE + V transpose ----
            with (
                tc.tile_pool(name="xw", bufs=1) as xw,
                tc.tile_pool(name="rtmp", bufs=2) as rtmp,
                tc.tile_pool(name="ps1", bufs=3, space="PSUM") as ps1,
                tc.tile_pool(name="tps", bufs=2, space="PSUM") as tps,
            ):
                xT_sb = [xw.tile([128, S], BF16, tag=f"x{kt}", name=f"x{kt}")
                         for kt in range(NKT)]
                wq_sb = [xw.tile([128, HQL * HD], BF16, tag=f"wq{kt}", name=f"wq{kt}")
                         for kt in range(NKT)]
                wk_sb = [xw.tile([128, HKL * HD], BF16, tag=f"wk{kt}", name=f"wk{kt}")
                         for kt in range(NKT)]
                wv_sb = [xw.tile([128, HKL * HD], BF16, tag=f"wv{kt}", name=f"wv{kt}")
                         for kt in range(NKT)]
                vt_sb = xw.tile([128, 2, S], BF16, name="vt_sb")

                # weights in run-consumption order on the scalar queue; x
                # half-tiles on the sync queue with the h0 halves first so
                # the first runs don't starve.
                for kt in range(NKT):
                    nc.scalar.dma_start(wk_sb[kt][:], wk_d[kt, :, :])
                for kt in range(NKT):
                    nc.scalar.dma_start(wq_sb[kt][:], wq_d[kt, :, :])
                for kt in range(NKT):
                    nc.scalar.dma_start(wv_sb[kt][:], wv_d[kt, :, :])
                nc.scalar.dma_start(cos_sb[:], cos_d[:])
                nc.scalar.dma_start(sin_sb[:], sin_d[:])
                nc.scalar.dma_start(mask_sb[:], mask_d[:])
                nc.scalar.dma_start(id_sb[:], id_d[:])
                nc.scalar.dma_start(wo_sb[:], wo_d[:])
                for h in range(2):
                    for kt in range(NKT):
                        hsl = slice(h * SH, (h + 1) * SH)
                        nc.sync.dma_start(xT_sb[kt][:, hsl], xT_d[kt, :, hsl])

                # warm the DVE against the table DMAs so later TTs carry a
                # single fresh cross-engine wait only.
                nc.vector.tensor_copy(warm_sb[:], cos_sb[0:1, 0:128])
                nc.vector.tensor_copy(warm_sb[:], sin_sb[0:1, 0:128])
                nc.vector.tensor_copy(warm_sb[:], mask_sb[0:1, 0:128])

                def rope_drain(ps, dst, hsl, tag):
                    qsb = rtmp.tile([128, SH], BF16, tag="evac", name=f"ev{tag}")
                    nc.scalar.copy(qsb[:], ps[:])
                    qsw = rtmp.tile([128, SH], BF16, tag="swap", name=f"sw{tag}")
                    nc.vector.tensor_copy(qsw[0:64, :], qsb[64:128, :])
                    nc.vector.tensor_copy(qsw[64:128, :], qsb[0:64, :])
                    a_t = rtmp.tile([128, SH], BF16, tag="ropeA", name=f"ra{tag}")
                    nc.vector.tensor_tensor(a_t[:], qsb[:], cos_sb[:, hsl], OP.mult)
                    b_t = rtmp.tile([128, SH], BF16, tag="ropeB", name=f"rb{tag}")
                    nc.vector.tensor_tensor(b_t[:], qsw[:], sin_sb[:, hsl], OP.mult)
                    nc.vector.tensor_tensor(dst, a_t[:], b_t[:], OP.add)

                # runs: pairs of head-tiles, h0-major.  Per kt the 4 MMs walk
                # 4 distinct PSUM banks (A.c0 A.c1 B.c0 B.c1) so each bank is
                # revisited at distance 4 and drains overlap fills.
                runs = []
                for h in range(2):
                    runs += [("k", 0, h), ("q", 0, h), ("q", 2, h), ("v", 0, h)]

                for kind, j0, h in runs:
                    hsl = slice(h * SH, (h + 1) * SH)
                    w_list = {"q": wq_sb, "k": wk_sb, "v": wv_sb}[kind]
                    psA = ps1.tile([128, SH], F32, tag="ps", name=f"psA_{kind}{j0}{h}")
                    psB = ps1.tile([128, SH], F32, tag="ps", name=f"psB_{kind}{j0}{h}")
                    for kt in range(NKT):
                        st, sp = (kt == 0), (kt == NKT - 1)
                        for ps, j in ((psA, j0), (psB, j0 + 1)):
                            for c in range(2):
                                nc.tensor.matmul(
                                    ps[:, c * 512:(c + 1) * 512],
                                    w_list[kt][:, j * HD:(j + 1) * HD],
                                    xT_sb[kt][:, h * SH + c * 512:
                                              h * SH + (c + 1) * 512],
                                    start=st, stop=sp,
                                    skip_group_check=True)
                    for ps, j in ((psA, j0), (psB, j0 + 1)):
                        if kind == "q":
                            rope_drain(ps, qt2[j // 2][:, j % 2, hsl], hsl,
                                       f"q{j}{h}")
                        elif kind == "k":
                            rope_drain(ps, kt2[:, j, hsl], hsl, f"k{j}{h}")
                        else:
                            nc.scalar.copy(vt_sb[:, j, hsl], ps[:])
                            tp = tps.tile([128, 8, 128], BF16, tag="tp",
                                          name=f"tp{j}{h}")
                            for blk in range(8):
                                nc.tensor.transpose(
                                    tp[:, blk, :],
                                    vt_sb[:, j, h * SH + blk * 128:
                                          h * SH + (blk + 1) * 128],
                                    id_sb[:])
                            nc.scalar.copy(
                                v_sb[:, h * 8:h * 8 + 8, j * HD:(j + 1) * HD],
                                tp[:])

            # ---------- phase 2: attention, softmax-pipelined ----------
            tasks = []
            for qc in range(NQC):
                for p in range(2):
                    nblk = 4 * qc + 4
                    for kb in range(nblk):
                        diag = kb >= 4 * qc
                        doff = (kb - 4 * qc) * 128 if diag else 0
                        tasks.append(dict(
                            p=p, qc=qc, kb=kb, diag=diag, doff=doff,
                            first=(kb == 0), last=(kb == nblk - 1)))

            with (
                tc.tile_pool(name="psS", bufs=2, space="PSUM") as psS,
                tc.tile_pool(name="psO", bufs=1, space="PSUM") as psO,
                tc.tile_pool(name="psZ", bufs=1, space="PSUM") as psZ,
                tc.tile_pool(name="epool", bufs=3) as epool,
                tc.tile_pool(name="opool", bufs=2) as opool,
                tc.tile_pool(name="rzpool", bufs=2) as rzpool,
            ):
                live = {}   # per-(p,qc) psum accumulators
                deferred = []

                def emit_scores(i, t):
                    p, qc, kb, doff = t["p"], t["qc"], t["kb"], t["doff"]
                    sps = psS.tile([128, 2, QC], F32, tag="sps", name=f"sps{i}")
                    t["sps"] = sps
                    for jj in range(2):
                        nc.tensor.matmul(
                            sps[:, jj, doff:],
                            kt2[:, p, kb * 128:(kb + 1) * 128],
                            qt2[p][:, jj, qc * QC + doff:(qc + 1) * QC],
                            start=True, stop=True, skip_group_check=True)

                def emit_expmask(i, t):
                    doff = t["doff"]
                    e2 = epool.tile([128, 2, QC], BF16, tag="e", name=f"e{i}")
                    t["e2"] = e2
                    sps = t["sps"]
                    if t["diag"]:
                        nc.scalar.activation(
                            e2[:, :, doff:doff + 128], sps[:, :, doff:doff + 128],
                            AF.Exp, scale=SCALE)
                        for jj in range(2):
                            nc.vector.tensor_tensor(
                                e2[:, jj, doff:doff + 128],
                                e2[:, jj, doff:doff + 128],
                                mask_sb[:], OP.mult)
                        if doff + 128 < QC:
                            nc.scalar.activation(
                                e2[:, :, doff + 128:], sps[:, :, doff + 128:],
                                AF.Exp, scale=SCALE)
                    else:
                        nc.scalar.activation(e2[:], sps[:], AF.Exp, scale=SCALE)

                def emit_pvz(i, t):
                    p, qc, kb, doff = t["p"], t["qc"], t["kb"], t["doff"]
                    if t["first"]:
                        live[(p, qc)] = (
                            psO.tile([128, 2, QC], F32, tag="ops", name=f"ops{i}"),
                            psZ.tile([128, 2, QC], F32, tag="zps", name=f"zps{i}"),
                        )
                    ops, zps = live[(p, qc)]
                    e2 = t["e2"]
                    for jj in range(2):
                        nc.tensor.matmul(
                            ops[:, jj, doff:], v_sb[:, kb, p * HD:(p + 1) * HD],
                            e2[:, jj, doff:], start=t["first"], stop=t["last"],
                            skip_group_check=True)
                    for jj in range(2):
                        nc.tensor.matmul(
                            zps[:, jj, doff:], ones_sb[:],
                            e2[:, jj, doff:], start=t["first"], stop=t["last"],
                            skip_group_check=True)

                def emit_tail(i, t):
                    p, qc = t["p"], t["qc"]
                    ops, zps = live.pop((p, qc))
                    o_un = opool.tile([128, 2, QC], BF16, tag="oun",
                                      name=f"oun{i}")
                    nc.scalar.copy(o_un[:], ops[:])
                    rz = rzpool.tile([128, 2, QC], F32, tag="rz", name=f"rz{i}")
                    nc.vector.reciprocal_approx_fast(rz[:], zps[:])

                    def norm(p=p, qc=qc, o_un=o_un, rz=rz):
                        nc.vector.tensor_tensor(
                            ot2[p][:, :, qc * QC:(qc + 1) * QC],
                            o_un[:], rz[:], OP.mult)
                    deferred.append((i + 2, norm))

                emit_scores(0, tasks[0])
                for i, t in enumerate(tasks):
                    if i + 1 < len(tasks):
                        emit_scores(i + 1, tasks[i + 1])
                    emit_expmask(i, t)
                    emit_pvz(i, t)
                    while deferred and deferred[0][0] <= i:
                        deferred.pop(0)[1]()
                    if t["last"]:
                        emit_tail(i, t)
                for _, fn in deferred:
                    fn()

            # ---------- phase 3: output projection ----------
            with (
                tc.tile_pool(name="ps3", bufs=2, space="PSUM") as ps3,
                tc.tile_pool(name="osb", bufs=2) as osb,
            ):
                for sb in range(NSB):
                    fps = ps3.tile([128, D], F32, tag="fps", name=f"fps{sb}")
                    for p in range(2):
                        for ii in range(2):
                            h = 2 * p + ii
                            lhs = ot2[p][:, ii, sb * 128:(sb + 1) * 128]
                            for dc in range(4):
                                nc.tensor.matmul(
                                    fps[:, dc * 512:(dc + 1) * 512], lhs,
                                    wo_sb[:, h, dc * 512:(dc + 1) * 512],
                                    start=(h == 0), stop=(h == HQL - 1),
                                    skip_group_check=True)
                    o_sb = osb.tile([128, D], BF16, tag="osb", name=f"osb{sb}")
                    nc.scalar.copy(o_sb[:], fps[:])
                    nc.sync.dma_start(out_d[sb, :, :], o_sb[:])

    nc.compile()
    return nc


_NC_CACHE = None


def _get_nc():
    global _NC_CACHE
    if _NC_CACHE is None:
        _NC_CACHE = _build_nc()
    return _NC_CACHE


def _rope_tables():
    inv = 1.0 / (ROPE_THETA ** (np.arange(0, HD, 2, dtype=np.float64) / HD))  # [64]
    t = np.arange(S, dtype=np.float64)
    ang = np.outer(inv, t)                      # [64, S]
    cos = np.cos(ang).astype(np.float32)
    sin = np.sin(ang).astype(np.float32)
    cos128 = np.concatenate([cos, cos], axis=0).astype(BF)  # [128, S]
    sinM = np.concatenate([-sin, sin], axis=0).astype(BF)
    return cos128, sinM


def prepare_inputs(x, wq, wk, wv, wo):
    """Build the 8 per-core input dicts from full inputs."""
    perm = np.concatenate([np.arange(0, HD, 2), np.arange(1, HD, 2)])
    cos128, sinM = _rope_tables()
    k_idx = np.arange(128)
    mask128 = (k_idx[:, None] <= k_idx[None, :]).astype(BF)
    ident = np.eye(128, dtype=np.float32).astype(BF)

    x = np.asarray(x, np.float32)
    wq = np.asarray(wq, np.float32).reshape(HQ, HD, D)[:, perm, :]
    wk = np.asarray(wk, np.float32).reshape(HKV, HD, D)[:, perm, :]
    wv = np.asarray(wv, np.float32)              # [HKV*HD, D]
    wo = np.asarray(wo, np.float32)              # [D, HQ*HD]

    in_maps = []
    for c in range(NCORES):
        b, hg = divmod(c, TP)
        qh = slice(hg * HQL, (hg + 1) * HQL)
        kh = slice(hg * HKL, (hg + 1) * HKL)
        xT = np.ascontiguousarray(x[b].T).reshape(NKT, 128, S).astype(BF)
        wq_t = np.ascontiguousarray(
            wq[qh].reshape(HQL * HD, D).T).reshape(NKT, 128, HQL * HD).astype(BF)
        wk_t = np.ascontiguousarray(
            wk[kh].reshape(HKL * HD, D).T).reshape(NKT, 128, HKL * HD).astype(BF)
        wv_t = np.ascontiguousarray(
            wv[hg * HKL * HD:(hg + 1) * HKL * HD].T
        ).reshape(NKT, 128, HKL * HD).astype(BF)
        wo_t = np.ascontiguousarray(
            wo[:, hg * HQL * HD:(hg + 1) * HQL * HD].T.reshape(HQL, 128, D)
            .transpose(1, 0, 2)).astype(BF)
        in_maps.append({
            "xT": xT, "wq_t": wq_t, "wk_t": wk_t, "wv_t": wv_t, "wo_t": wo_t,
            "cos128": cos128, "sinM": sinM, "mask128": mask128,
            "ident128": ident,
        })
    return in_maps


def _install_ntff_hook():
    """The agent image's antenv lacks axon_hooks; synthesize it so
    run_bass_kernel_spmd(trace=True) can capture NTFF profiles."""
    import sys as _sys
    import types, contextlib, ctypes

    if "antenv.axon_hooks" in _sys.modules:
        return
    so_path = "/opt/axon/libaxon_pjrt.so"
    lib = ctypes.CDLL(so_path)
    if not hasattr(lib, "axon_start_nrt_profile"):
        return
    lib.axon_start_nrt_profile.argtypes = [ctypes.POINTER(ctypes.c_int64),
                                           ctypes.c_size_t]
    lib.axon_start_nrt_profile.restype = ctypes.c_int64
    lib.axon_stop_nrt_profile.argtypes = [ctypes.c_char_p]
    lib.axon_stop_nrt_profile.restype = ctypes.c_int64

    @contextlib.contextmanager
    def _hook(output_dir, device_ids):
        import jax
        jax.devices()
        if device_ids:
            ids = (ctypes.c_int64 * len(device_ids))(*device_ids)
            rc = lib.axon_start_nrt_profile(ids, len(device_ids))
        else:
            rc = lib.axon_start_nrt_profile(None, 0)
        if rc != 0:
            raise RuntimeError(f"axon_start_nrt_profile rc={rc}")
        try:
            yield
        finally:
            n = lib.axon_stop_nrt_profile(str(output_dir).encode())
            print(f"ntff profile: {n} file(s) written to {output_dir}",
                  file=_sys.stderr)

    mod = types.ModuleType("antenv.axon_hooks")
    mod.get_axon_ntff_profile_hook = lambda: _hook
    mod.set_axon_ntff_profile_hook = lambda h: None
    _sys.modules["antenv.axon_hooks"] = mod
    try:
        import antenv
        antenv.axon_hooks = mod
    except ImportError:
        pass


def kernel(x, wq, wk, wv, wo, _trace=False, _trace_cores=None):
    in_maps = prepare_inputs(x, wq, wk, wv, wo)
    if _trace:
        _install_ntff_hook()
    nc = _get_nc()
    res = run_bass_kernel_spmd(
        nc, in_maps, core_ids=list(range(NCORES)),
        trace=_trace, trace_cores=_trace_cores)
    out = np.zeros((B, S, D), np.float32)
    for c in range(NCORES):
        b = c // TP
        out[b] += res.results[c]["out"].reshape(S, D).astype(np.float32)
    kernel.last_results = res
    return out


if __name__ == "__main__":
    rng = np.random.default_rng(0)
    x = rng.standard_normal((B, S, D), dtype=np.float32)
    sc = 1.0 / np.sqrt(D)
    wq = (rng.standard_normal((HQ * HD, D), dtype=np.float32) * sc)
    wk = (rng.standard_normal((HKV * HD, D), dtype=np.float32) * sc)
    wv = (rng.standard_normal((HKV * HD, D), dtype=np.float32) * sc)
    wo = (rng.standard_normal((D, HQ * HD), dtype=np.float32) * sc)
    out = kernel(x, wq, wk, wv, wo)
    print("ran", out.shape, out.dtype, float(np.abs(out).mean()))


# revision 13
# speedup vs baseline: 1.6432x; 1.0415x over previous
"""Causal self-attention (RoPE, GQA) on 8 Trainium2 NeuronCores — v2.

Sharding: 2-way data-parallel over batch x 4-way tensor-parallel over heads.
Core c handles batch c//4 and head-group c%4 (4 q-heads, 2 kv-heads).
Each core computes its partial output projection (wo row-shard); the host
sums the 4 partials per batch (the "all-reduce" happens in the unshard step).

v2 changes vs v1 (518us):
  - everything bf16 (x, weights): 1 cyc/row matmuls + FWL weight loads,
    half the DMA bytes.
  - moving dim 1024 everywhere possible (bf16 max), halving MM count.
  - GQA head pairing: the 2 q-heads sharing a kv head are batched in one
    matmul (rhs [128, 2, n]) for scores / PV / Z.
  - attention is software-pipelined: scores(i+1) issued before PV/Z(i) so
    the PE never waits for the exp (ACT) of the current block.
  - Z (softmax denominator) via ones-MATRIX matmul -> [128, 2, 512] psum:
    the denominator is broadcast to all partitions by the PE itself, so
    normalization is one DVE reciprocal + one TT (no K=1 broadcast matmul).
  - V computed transposed like K (good LDWEIGHTS amortization) then
    PE-transposed in 128x128 blocks (cheap) into PV layout.
  - output projection as a dense phase-3 with full PSUM double-buffering.

Layouts:
  - x fed as xT [D, S] bf16; Q/K produced as [head_dim, S] via lhsT=weight
    slice; V^T the same way, then transposed.
  - RoPE: weight rows pre-permuted (even components -> rows 0:64, odd ->
    64:128) so rotation is elementwise on row halves.
  - scores computed transposed [k, q-pair] so the softmax numerator feeds
    the PV matmul directly.
"""

import sys
import numpy as np
import ml_dtypes

sys.path.insert(0, "/opt/trn_rl_repo")

import concourse.bass as bass
import concourse.bacc as bacc
import concourse.mybir as mybir
from concourse import tile
from concourse.bass_utils import run_bass_kernel_spmd

F32 = mybir.dt.float32
BF16 = mybir.dt.bfloat16
AF = mybir.ActivationFunctionType
OP = mybir.AluOpType

B, S, D = 2, 2048, 2048
HQ, HKV, HD = 16, 8, 128
ROPE_THETA = 10000.0
NCORES, TP = 8, 4
HQL, HKL = HQ // TP, HKV // TP        # 4 q heads, 2 kv heads per core
NKT = D // 128                        # 16 contraction tiles
NSB = S // 128                        # 16 s-blocks
QC = 512                              # q-chunk width in attention
NQC = S // QC                         # 4 q chunks
SH = S // 2                           # phase-1 S-half (moving dim 1024)
SCALE = 1.0 / float(np.sqrt(HD))
BF = ml_dtypes.bfloat16


def _build_nc():
    nc = bacc.Bacc("TRN2", target_bir_lowering=False)

    xT_d = nc.dram_tensor("xT", [NKT, 128, S], BF16, kind="ExternalInput")
    wq_d = nc.dram_tensor("wq_t", [NKT, 128, HQL * HD], BF16, kind="ExternalInput")
    wk_d = nc.dram_tensor("wk_t", [NKT, 128, HKL * HD], BF16, kind="ExternalInput")
    wv_d = nc.dram_tensor("wv_t", [NKT, 128, HKL * HD], BF16, kind="ExternalInput")
    wo_d = nc.dram_tensor("wo_t", [128, HQL, D], BF16, kind="ExternalInput")
    cos_d = nc.dram_tensor("cos128", [128, S], BF16, kind="ExternalInput")
    sin_d = nc.dram_tensor("sinM", [128, S], BF16, kind="ExternalInput")
    mask_d = nc.dram_tensor("mask128", [128, 128], BF16, kind="ExternalInput")
    id_d = nc.dram_tensor("ident128", [128, 128], BF16, kind="ExternalInput")
    out_d = nc.dram_tensor("out", [NSB, 128, D], BF16, kind="ExternalOutput")

    with tile.TileContext(nc) as tc:
        with tc.tile_pool(name="res", bufs=1) as res:
            # residents that live across phases
            qt2 = [res.tile([128, 2, S], BF16, name=f"qt2_{p}") for p in range(2)]
            kt2 = res.tile([128, 2, S], BF16, name="kt2")
            v_sb = res.tile([128, NSB, HKL * HD], BF16, name="v_sb")
            ot2 = [res.tile([128, 2, S], BF16, name=f"ot2_{p}") for p in range(2)]
            wo_sb = res.tile([128, HQL, D], BF16, name="wo_sb")
            cos_sb = res.tile([128, S], BF16, name="cos_sb")
            sin_sb = res.tile([128, S], BF16, name="sin_sb")
            mask_sb = res.tile([128, 128], BF16, name="mask_sb")
            id_sb = res.tile([128, 128], BF16, name="id_sb")
            ones_sb = res.tile([128, 128], BF16, name="ones_sb")
            warm_sb = res.tile([1, 128], BF16, name="warm_sb")

            nc.vector.memset(ones_sb[:], 1.0)

            # ---------- phase 1: QKV^T projection + RoPE + V transpose ----
            with (
                tc.tile_pool(name="xw", bufs=1) as xw,
                tc.tile_pool(name="rtmp", bufs=2) as rtmp,
                tc.tile_pool(name="ps1", bufs=3, space="PSUM") as ps1,
                tc.tile_pool(name="tps", bufs=2, space="PSUM") as tps,
            ):
                xT_sb = [xw.tile([128, S], BF16, tag=f"x{kt}", name=f"x{kt}")
                         for kt in range(NKT)]
                wq_sb = [xw.tile([128, HQL * HD], BF16, tag=f"wq{kt}", name=f"wq{kt}")
                         for kt in range(NKT)]
                wk_sb = [xw.tile([128, HKL * HD], BF16, tag=f"wk{kt}", name=f"wk{kt}")
                         for kt in range(NKT)]
                wv_sb = [xw.tile([128, HKL * HD], BF16, tag=f"wv{kt}", name=f"wv{kt}")
                         for kt in range(NKT)]
                vt_sb = xw.tile([128, 2, S], BF16, name="vt_sb")

                # weights in run-consumption order on the scalar queue; x
                # half-tiles on the sync queue with the h0 halves first so
                # the first runs don't starve.
                for kt in range(NKT):
                    nc.gpsimd.dma_start(wk_sb[kt][:], wk_d[kt, :, :])
                for kt in range(NKT):
                    nc.gpsimd.dma_start(wq_sb[kt][:], wq_d[kt, :, :])
                for kt in range(NKT):
                    nc.gpsimd.dma_start(wv_sb[kt][:], wv_d[kt, :, :])
                nc.gpsimd.dma_start(cos_sb[:], cos_d[:])
                nc.gpsimd.dma_start(sin_sb[:], sin_d[:])
                nc.gpsimd.dma_start(mask_sb[:], mask_d[:])
                nc.gpsimd.dma_start(id_sb[:], id_d[:])
                nc.gpsimd.dma_start(wo_sb[:], wo_d[:])
                for h in range(2):
                    for kt in range(NKT):
                        hsl = slice(h * SH, (h + 1) * SH)
                        nc.sync.dma_start(xT_sb[kt][:, hsl], xT_d[kt, :, hsl])

                # warm the DVE against the table DMAs so later TTs carry a
                # single fresh cross-engine wait only.
                nc.vector.tensor_copy(warm_sb[:], cos_sb[0:1, 0:128])
                nc.vector.tensor_copy(warm_sb[:], sin_sb[0:1, 0:128])
                nc.vector.tensor_copy(warm_sb[:], mask_sb[0:1, 0:128])

                def rope_drain(ps, dst, hsl, tag):
                    qsb = rtmp.tile([128, SH], BF16, tag="evac", name=f"ev{tag}")
                    nc.scalar.copy(qsb[:], ps[:])
                    qsw = rtmp.tile([128, SH], BF16, tag="swap", name=f"sw{tag}")
                    nc.vector.tensor_copy(qsw[0:64, :], qsb[64:128, :])
                    nc.vector.tensor_copy(qsw[64:128, :], qsb[0:64, :])
                    a_t = rtmp.tile([128, SH], BF16, tag="ropeA", name=f"ra{tag}")
                    nc.vector.tensor_tensor(a_t[:], qsb[:], cos_sb[:, hsl], OP.mult)
                    b_t = rtmp.tile([128, SH], BF16, tag="ropeB", name=f"rb{tag}")
                    nc.vector.tensor_tensor(b_t[:], qsw[:], sin_sb[:, hsl], OP.mult)
                    nc.vector.tensor_tensor(dst, a_t[:], b_t[:], OP.add)

                # runs: pairs of head-tiles, h0-major.  Per kt the 4 MMs walk
                # 4 distinct PSUM banks (A.c0 A.c1 B.c0 B.c1) so each bank is
                # revisited at distance 4 and drains overlap fills.
                runs = []
                for h in range(2):
                    runs += [("k", 0, h), ("q", 0, h), ("q", 2, h), ("v", 0, h)]

                for kind, j0, h in runs:
                    hsl = slice(h * SH, (h + 1) * SH)
                    w_list = {"q": wq_sb, "k": wk_sb, "v": wv_sb}[kind]
                    psA = ps1.tile([128, SH], F32, tag="ps", name=f"psA_{kind}{j0}{h}")
                    psB = ps1.tile([128, SH], F32, tag="ps", name=f"psB_{kind}{j0}{h}")
                    for kt in range(NKT):
                        st, sp = (kt == 0), (kt == NKT - 1)
                        for ps, j in ((psA, j0), (psB, j0 + 1)):
                            for c in range(2):
                                nc.tensor.matmul(
                                    ps[:, c * 512:(c + 1) * 512],
                                    w_list[kt][:, j * HD:(j + 1) * HD],
                                    xT_sb[kt][:, h * SH + c * 512:
                                              h * SH + (c + 1) * 512],
                                    start=st, stop=sp,
                                    skip_group_check=True)
                    for ps, j in ((psA, j0), (psB, j0 + 1)):
                        if kind == "q":
                            rope_drain(ps, qt2[j // 2][:, j % 2, hsl], hsl,
                                       f"q{j}{h}")
                        elif kind == "k":
                            rope_drain(ps, kt2[:, j, hsl], hsl, f"k{j}{h}")
                        else:
                            nc.scalar.copy(vt_sb[:, j, hsl], ps[:])
                            tp = tps.tile([128, 8, 128], BF16, tag="tp",
                                          name=f"tp{j}{h}")
                            for blk in range(8):
                                nc.tensor.transpose(
                                    tp[:, blk, :],
                                    vt_sb[:, j, h * SH + blk * 128:
                                          h * SH + (blk + 1) * 128],
                                    id_sb[:])
                            nc.scalar.copy(
                                v_sb[:, h * 8:h * 8 + 8, j * HD:(j + 1) * HD],
                                tp[:])

            # ---------- phase 2: attention, softmax-pipelined ----------
            tasks = []
            for qc in range(NQC):
                for p in range(2):
                    nblk = 4 * qc + 4
                    for kb in range(nblk):
                        diag = kb >= 4 * qc
                        doff = (kb - 4 * qc) * 128 if diag else 0
                        tasks.append(dict(
                            p=p, qc=qc, kb=kb, diag=diag, doff=doff,
                            first=(kb == 0), last=(kb == nblk - 1)))

            with (
                tc.tile_pool(name="psS", bufs=2, space="PSUM") as psS,
                tc.tile_pool(name="psO", bufs=1, space="PSUM") as psO,
                tc.tile_pool(name="psZ", bufs=1, space="PSUM") as psZ,
                tc.tile_pool(name="epool", bufs=3) as epool,
                tc.tile_pool(name="opool", bufs=2) as opool,
                tc.tile_pool(name="rzpool", bufs=2) as rzpool,
            ):
                live = {}   # per-(p,qc) psum accumulators
                deferred = []

                def emit_scores(i, t):
                    p, qc, kb, doff = t["p"], t["qc"], t["kb"], t["doff"]
                    sps = psS.tile([128, 2, QC], F32, tag="sps", name=f"sps{i}")
                    t["sps"] = sps
                    for jj in range(2):
                        nc.tensor.matmul(
                            sps[:, jj, doff:],
                            kt2[:, p, kb * 128:(kb + 1) * 128],
                            qt2[p][:, jj, qc * QC + doff:(qc + 1) * QC],
                            start=True, stop=True, skip_group_check=True)

                def emit_expmask(i, t):
                    doff = t["doff"]
                    e2 = epool.tile([128, 2, QC], BF16, tag="e", name=f"e{i}")
                    t["e2"] = e2
                    sps = t["sps"]
                    if t["diag"]:
                        nc.scalar.activation(
                            e2[:, :, doff:doff + 128], sps[:, :, doff:doff + 128],
                            AF.Exp, scale=SCALE)
                        for jj in range(2):
                            nc.vector.tensor_tensor(
                                e2[:, jj, doff:doff + 128],
                                e2[:, jj, doff:doff + 128],
                                mask_sb[:], OP.mult)
                        if doff + 128 < QC:
                            nc.scalar.activation(
                                e2[:, :, doff + 128:], sps[:, :, doff + 128:],
                                AF.Exp, scale=SCALE)
                    else:
                        nc.scalar.activation(e2[:], sps[:], AF.Exp, scale=SCALE)

                def emit_pvz(i, t):
                    p, qc, kb, doff = t["p"], t["qc"], t["kb"], t["doff"]
                    if t["first"]:
                        live[(p, qc)] = (
                            psO.tile([128, 2, QC], F32, tag="ops", name=f"ops{i}"),
                            psZ.tile([128, 2, QC], F32, tag="zps", name=f"zps{i}"),
                        )
                    ops, zps = live[(p, qc)]
                    e2 = t["e2"]
                    for jj in range(2):
                        nc.tensor.matmul(
                            ops[:, jj, doff:], v_sb[:, kb, p * HD:(p + 1) * HD],
                            e2[:, jj, doff:], start=t["first"], stop=t["last"],
                            skip_group_check=True)
                    for jj in range(2):
                        nc.tensor.matmul(
                            zps[:, jj, doff:], ones_sb[:],
                            e2[:, jj, doff:], start=t["first"], stop=t["last"],
                            skip_group_check=True)

                def emit_tail(i, t):
                    p, qc = t["p"], t["qc"]
                    ops, zps = live.pop((p, qc))
                    o_un = opool.tile([128, 2, QC], BF16, tag="oun",
                                      name=f"oun{i}")
                    nc.scalar.copy(o_un[:], ops[:])
                    rz = rzpool.tile([128, 2, QC], F32, tag="rz", name=f"rz{i}")
                    nc.vector.reciprocal_approx_fast(rz[:], zps[:])

                    def norm(p=p, qc=qc, o_un=o_un, rz=rz):
                        nc.vector.tensor_tensor(
                            ot2[p][:, :, qc * QC:(qc + 1) * QC],
                            o_un[:], rz[:], OP.mult)
                    deferred.append((i + 2, norm))

                emit_scores(0, tasks[0])
                for i, t in enumerate(tasks):
                    if i + 1 < len(tasks):
                        emit_scores(i + 1, tasks[i + 1])
                    emit_expmask(i, t)
                    emit_pvz(i, t)
                    while deferred and deferred[0][0] <= i:
                        deferred.pop(0)[1]()
                    if t["last"]:
                        emit_tail(i, t)
                for _, fn in deferred:
                    fn()

            # ---------- phase 3: output projection ----------
            with (
                tc.tile_pool(name="ps3", bufs=2, space="PSUM") as ps3,
                tc.tile_pool(name="osb", bufs=2) as osb,
            ):
                for sb in range(NSB):
                    fps = ps3.tile([128, D], F32, tag="fps", name=f"fps{sb}")
                    for p in range(2):
                        for ii in range(2):
                            h = 2 * p + ii
                            lhs = ot2[p][:, ii, sb * 128:(sb + 1) * 128]
                            for dc in range(4):
                                nc.tensor.matmul(
                                    fps[:, dc * 512:(dc + 1) * 512], lhs,
                                    wo_sb[:, h, dc * 512:(dc + 1) * 512],
                                    start=(h == 0), stop=(h == HQL - 1),
                                    skip_group_check=True)
                    o_sb = osb.tile([128, D], BF16, tag="osb", name=f"osb{sb}")
                    nc.scalar.copy(o_sb[:], fps[:])
                    nc.sync.dma_start(out_d[sb, :, :], o_sb[:])

    nc.compile()
    return nc


_NC_CACHE = None


def _get_nc():
    global _NC_CACHE
    if _NC_CACHE is None:
        _NC_CACHE = _build_nc()
    return _NC_CACHE


def _rope_tables():
    inv = 1.0 / (ROPE_THETA ** (np.arange(0, HD, 2, dtype=np.float64) / HD))  # [64]
    t = np.arange(S, dtype=np.float64)
    ang = np.outer(inv, t)                      # [64, S]
    cos = np.cos(ang).astype(np.float32)
    sin = np.sin(ang).astype(np.float32)
    cos128 = np.concatenate([cos, cos], axis=0).astype(BF)  # [128, S]
    sinM = np.concatenate([-sin, sin], axis=0).astype(BF)
    return cos128, sinM


def prepare_inputs(x, wq, wk, wv, wo):
    """Build the 8 per-core input dicts from full inputs."""
    perm = np.concatenate([np.arange(0, HD, 2), np.arange(1, HD, 2)])
    cos128, sinM = _rope_tables()
    k_idx = np.arange(128)
    mask128 = (k_idx[:, None] <= k_idx[None, :]).astype(BF)
    ident = np.eye(128, dtype=np.float32).astype(BF)

    x = np.asarray(x, np.float32)
    wq = np.asarray(wq, np.float32).reshape(HQ, HD, D)[:, perm, :]
    wk = np.asarray(wk, np.float32).reshape(HKV, HD, D)[:, perm, :]
    wv = np.asarray(wv, np.float32)              # [HKV*HD, D]
    wo = np.asarray(wo, np.float32)              # [D, HQ*HD]

    in_maps = []
    for c in range(NCORES):
        b, hg = divmod(c, TP)
        qh = slice(hg * HQL, (hg + 1) * HQL)
        kh = slice(hg * HKL, (hg + 1) * HKL)
        xT = np.ascontiguousarray(x[b].T).reshape(NKT, 128, S).astype(BF)
        wq_t = np.ascontiguousarray(
            wq[qh].reshape(HQL * HD, D).T).reshape(NKT, 128, HQL * HD).astype(BF)
        wk_t = np.ascontiguousarray(
            wk[kh].reshape(HKL * HD, D).T).reshape(NKT, 128, HKL * HD).astype(BF)
        wv_t = np.ascontiguousarray(
            wv[hg * HKL * HD:(hg + 1) * HKL * HD].T
        ).reshape(NKT, 128, HKL * HD).astype(BF)
        wo_t = np.ascontiguousarray(
            wo[:, hg * HQL * HD:(hg + 1) * HQL * HD].T.reshape(HQL, 128, D)
            .transpose(1, 0, 2)).astype(BF)
        in_maps.append({
            "xT": xT, "wq_t": wq_t, "wk_t": wk_t, "wv_t": wv_t, "wo_t": wo_t,
            "cos128": cos128, "sinM": sinM, "mask128": mask128,
            "ident128": ident,
        })
    return in_maps


def _install_ntff_hook():
    """The agent image's antenv lacks axon_hooks; synthesize it so
    run_bass_kernel_spmd(trace=True) can capture NTFF profiles."""
    import sys as _sys
    import types, contextlib, ctypes

    if "antenv.axon_hooks" in _sys.modules:
        return
    so_path = "/opt/axon/libaxon_pjrt.so"
    lib = ctypes.CDLL(so_path)
    if not hasattr(lib, "axon_start_nrt_profile"):
        return
    lib.axon_start_nrt_profile.argtypes = [ctypes.POINTER(ctypes.c_int64),
                                           ctypes.c_size_t]
    lib.axon_start_nrt_profile.restype = ctypes.c_int64
    lib.axon_stop_nrt_profile.argtypes = [ctypes.c_char_p]
    lib.axon_stop_nrt_profile.restype = ctypes.c_int64

    @contextlib.contextmanager
    def _hook(output_dir, device_ids):
        import jax
        jax.devices()
        if device_ids:
            ids = (ctypes.c_int64 * len(device_ids))(*device_ids)
            rc = lib.axon_start_nrt_profile(ids, len(device_ids))
        else:
            rc = lib.axon_start_nrt_profile(None, 0)
        if rc != 0:
            raise RuntimeError(f"axon_start_nrt_profile rc={rc}")
        try:
            yield
        finally:
            n = lib.axon_stop_nrt_profile(str(output_dir).encode())
            print(f"ntff profile: {n} file(s) written to {output_dir}",
                  file=_sys.stderr)

    mod = types.ModuleType("antenv.axon_hooks")
    mod.get_axon_ntff_profile_hook = lambda: _hook
    mod.set_axon_ntff_profile_hook = lambda h: None
    _sys.modules["antenv.axon_hooks"] = mod
    try:
        import antenv
        antenv.axon_hooks = mod
    except ImportError:
        pass


def kernel(x, wq, wk, wv, wo, _trace=False, _trace_cores=None):
    in_maps = prepare_inputs(x, wq, wk, wv, wo)
    if _trace:
        _install_ntff_hook()
    nc = _get_nc()
    res = run_bass_kernel_spmd(
        nc, in_maps, core_ids=list(range(NCORES)),
        trace=_trace, trace_cores=_trace_cores)
    out = np.zeros((B, S, D), np.float32)
    for c in range(NCORES):
        b = c // TP
        out[b] += res.results[c]["out"].reshape(S, D).astype(np.float32)
    kernel.last_results = res
    return out


if __name__ == "__main__":
    rng = np.random.default_rng(0)
    x = rng.standard_normal((B, S, D), dtype=np.float32)
    sc = 1.0 / np.sqrt(D)
    wq = (rng.standard_normal((HQ * HD, D), dtype=np.float32) * sc)
    wk = (rng.standard_normal((HKV * HD, D), dtype=np.float32) * sc)
    wv = (rng.standard_normal((HKV * HD, D), dtype=np.float32) * sc)
    wo = (rng.standard_normal((D, HQ * HD), dtype=np.float32) * sc)
    out = kernel(x, wq, wk, wv, wo)
    print("ran", out.shape, out.dtype, float(np.abs(out).mean()))
